# revision 31
# baseline (speedup 1.0000x reference)
"""Trainium2 Bass kernel for nn_MAB_72911364817388 (dense transformer block).

Reference computation (per batch element b):
    q = Q @ Wq + bq ; k = K @ Wk + bk ; v = K @ Wv + bv        (1024x512 @ 512x512)
    scores = einsum("qhd,khd->hqk", qh, kh) / sqrt(512)
    scores = where(mask==0, -1e4, scores); attn = softmax(scores, axis=k)
    oh = qh + attn @ vh ; O = LN0(oh) ; O = O + relu(O @ Wo + bo) ; O = LN1(O)

Strategy: pure data-parallel over batch B=8 -> one batch element per core.
All on-chip activations are kept "d-major" (feature dim on partitions),
which makes attention, the FC layer and per-feature bias/scale natural.
LayerNorm reductions over d (the partition axis) are done with ones-vector
matmuls on the TensorEngine, which also replicates the stats across all
128 partitions for free.

Softmax is computed unnormalized: p = exp(s/sqrt(512) + maskbias), where
maskbias is -100 for masked keys (exp underflows to ~3e-44, matching the
reference's -1e4 masking after normalization to < 1e-40 relative).  The
denominator comes from an extra ones-column matmul and is divided out
after attn @ v.  Scores never exceed ~±6 so no max-subtraction is needed.

Matmuls run as float32r (fp32 storage, reduced-precision PE mode, 4x
faster than plain fp32; ~1.6e-4 rel err per matmul measured on HW).  The
attention core (scores and attn@v) uses bf16 operands, which stream at
the same rate but with much cheaper self-loading weights (~237ns vs
~329ns per 512-wide matmul measured).  Keys are host-compacted: unmasked
keys are moved to the front (softmax is permutation-invariant; fully
masked keys contribute exactly zero), so the attention core processes
~640 of 1024 keys for the usual p=0.5 masks.  Measured end-to-end
relative error vs the jax reference: ~6e-4 (scale-relative max).
"""

import numpy as np

import concourse.bass as bass
import concourse.mybir as mybir
import concourse.tile as tile
from concourse import bacc, bass_utils
from concourse.masks import make_identity

# Problem shapes (hardcoded per contract).
B = 8
NQ = NK = 1024
D = 512  # DQ = DK = DV
H = 8
HD = 64
P = 128
EPS = 1e-5
N_CORES = 8

DO = D // P  # 4   d-major partition groups
NO = NQ // P  # 8  q/k-major partition groups
QC = NQ // 512  # 2 free-dim chunks of 512

F32 = mybir.dt.float32
BF16 = mybir.dt.bfloat16
MM_DT = mybir.dt.float32r

AF = mybir.ActivationFunctionType
OP = mybir.AluOpType


def _mm(a):
    """Bitcast an fp32 AP to the matmul dtype (consumer side)."""
    return a.bitcast(MM_DT) if MM_DT != F32 else a


def _mo(a):
    """Bitcast a producer's out AP to the matmul dtype, so the BIR verifier
    sees matmul inputs as produced-rounded fp32r."""
    return a.bitcast(MM_DT) if MM_DT != F32 else a


def build_program(repeat: int = 1, apply_g0b0: bool = True,
                  apply_g1b1: bool = True, nkb: int = NO):
    """Build the per-core Bass program for nkb 128-row key blocks.

    The host compacts unmasked keys to the front (softmax is permutation-
    invariant over keys, and fully-masked keys contribute exactly 0), so
    nkb is usually ceil(max_unmasked/128) ~ 5 instead of 8."""
    nc = bacc.Bacc("TRN2", target_bir_lowering=False, debug=False,
                   num_devices=N_CORES)

    NKC = nkb * P
    Qd = nc.dram_tensor("Q", [NQ, D], F32, kind="ExternalInput").ap()
    Kd = nc.dram_tensor("K", [NKC, D], F32, kind="ExternalInput").ap()
    Md = nc.dram_tensor("attention_mask", [NKC], mybir.dt.int32,
                        kind="ExternalInput").ap()
    Wqd = nc.dram_tensor("Wq", [D, D], F32, kind="ExternalInput").ap()
    Wkd = nc.dram_tensor("Wk", [D, D], F32, kind="ExternalInput").ap()
    Wvd = nc.dram_tensor("Wv", [D, D], F32, kind="ExternalInput").ap()
    Wod = nc.dram_tensor("Wo", [D, D], F32, kind="ExternalInput").ap()
    bqd = nc.dram_tensor("bq", [D], F32, kind="ExternalInput").ap()
    bkd = nc.dram_tensor("bk", [D], F32, kind="ExternalInput").ap()
    bvd = nc.dram_tensor("bv", [D], F32, kind="ExternalInput").ap()
    bod = nc.dram_tensor("bo", [D], F32, kind="ExternalInput").ap()
    g0d = nc.dram_tensor("g0", [D], F32, kind="ExternalInput").ap()
    b0d = nc.dram_tensor("b0", [D], F32, kind="ExternalInput").ap()
    g1d = nc.dram_tensor("g1", [D], F32, kind="ExternalInput").ap()
    b1d = nc.dram_tensor("b1", [D], F32, kind="ExternalInput").ap()
    # Output is O^T (d-major); the host transposes back.
    OTd = nc.dram_tensor("OT", [D, NQ], F32, kind="ExternalOutput").ap()

    with tile.TileContext(nc) as tc:
        def body():
            _build_body(nc, tc,
                        Qd, Kd, Md, Wqd, Wkd, Wvd, Wod,
                        bqd, bkd, bvd, bod, g0d, b0d, g1d, b1d, OTd,
                        apply_g0b0, apply_g1b1, nkb)

        if repeat == 1:
            body()
        else:
            with tc.For_i(0, repeat, 1,
                          hint_engines=(mybir.EngineType.PE,
                                        mybir.EngineType.Activation,
                                        mybir.EngineType.DVE,
                                        mybir.EngineType.SP,
                                        mybir.EngineType.Pool)):
                body()

    nc.compile()
    return nc


def _build_body(nc, tc, Qd, Kd, Md, Wqd, Wkd, Wvd, Wod,
                bqd, bkd, bvd, bod, g0d, b0d, g1d, b1d, OTd,
                apply_g0b0, apply_g1b1, nkb):
    f32 = F32
    NKC = nkb * P
    kchunks = []
    off = 0
    while off < NKC:
        w = min(512, NKC - off)
        kchunks.append((off, w))
        off += w
    import contextlib
    ctx = contextlib.ExitStack()
    with ctx:
        consts = ctx.enter_context(tc.tile_pool(name="consts", bufs=1))
        bigs = ctx.enter_context(tc.tile_pool(name="bigs", bufs=1))
        small = ctx.enter_context(tc.tile_pool(name="small", bufs=3))

        # ---------- constants ----------
        ident = consts.tile([P, P], f32)
        make_identity(nc, ident)
        ones_stage = consts.tile([P, 512], f32)
        nc.vector.memset(ones_stage, 1.0)
        ones_pp = consts.tile([P, P], f32)   # all-ones for LN stat matmuls
        nc.vector.tensor_copy(out=_mo(ones_pp), in_=ones_stage[:, :P])
        ones_row = consts.tile([1, 512], f32)  # ones moving-vector for bias rank-1
        nc.vector.tensor_copy(out=_mo(ones_row), in_=ones_stage[:1, :])
        epsT = consts.tile([P, 1], f32)
        nc.vector.memset(epsT, EPS)

        # ---------- phase A: load + transpose Q, K; weight DMAs interleave
        Wq = consts.tile([P, DO, D], f32)
        Wk = consts.tile([P, DO, D], f32)
        Wv = consts.tile([P, DO, D], f32)
        Wo = consts.tile([P, DO, D], f32)
        QT = bigs.tile([P, DO, NQ], f32, tag="buf_qt")
        KT = bigs.tile([P, DO, NKC], f32, tag="buf_kt")
        with tc.tile_pool(name="ps_tr", bufs=4, space="PSUM") as ps_tr, \
             tc.tile_pool(name="rawp", bufs=3) as rawp:
            # batched raw loads: fewer, bigger DMAs (latency amortized)
            q0 = rawp.tile([P, 4, D], f32, tag="qraw")
            nc.sync.dma_start(out=q0,
                              in_=Qd[:512, :].rearrange("(j p) d -> p j d", p=P))
            nc.sync.dma_start(out=_mo(Wq),
                              in_=_mo(Wqd.rearrange("(o p) n -> p o n", p=P)))
            kr = rawp.tile([P, nkb, D], f32, tag="kraw")
            nc.sync.dma_start(out=kr,
                              in_=Kd.rearrange("(j p) d -> p j d", p=P))
            q1 = rawp.tile([P, 4, D], f32, tag="qraw")
            nc.sync.dma_start(out=q1,
                              in_=Qd[512:, :].rearrange("(j p) d -> p j d", p=P))
            nc.sync.dma_start(out=_mo(Wk),
                              in_=_mo(Wkd.rearrange("(o p) n -> p o n", p=P)))
            nc.sync.dma_start(out=_mo(Wv),
                              in_=_mo(Wvd.rearrange("(o p) n -> p o n", p=P)))
            nc.sync.dma_start(out=_mo(Wo),
                              in_=_mo(Wod.rearrange("(o p) n -> p o n", p=P)))

            worklist = [(j, 0, q0) for j in range(4)]
            worklist += [(j, 1, kr) for j in range(nkb)]
            worklist += [(j + 4, 0, q1) for j in range(4)]
            for no, sd, rawt in worklist:
                dstT, ceng = ((QT, 0), (KT, 1))[sd]
                raw = rawt[:, no % 4 if sd == 0 else no, :]
                ps = ps_tr.tile([P, 512], f32, tag="trps")
                for do in range(DO):
                    nc.tensor.transpose(ps[:, do * P:(do + 1) * P],
                                        raw[:, do * P:(do + 1) * P], ident)
                dst = dstT[:, :, no * P:(no + 1) * P]
                psv = ps.rearrange("p (a b) -> p a b", b=P)
                if (no + ceng) % 2 == 0:
                    nc.vector.tensor_copy(out=_mo(dst), in_=psv)
                else:
                    nc.scalar.copy(out=_mo(dst), in_=psv)

        # bias rows [1, 512]
        def load_row(ap):
            t = consts.tile([1, 512], f32)
            nc.sync.dma_start(out=_mo(t), in_=_mo(ap[None, :]))
            return t
        bv_r = load_row(bvd)

        # LN scale/shift per-partition columns [P, DO] (only if non-identity)
        def load_colT(ap, pspool):
            # [512] -> sbuf [4,128] -> PE transpose -> [128,4]
            nat = small.tile([DO, P], f32, tag="lncol_nat")
            nc.sync.dma_start(out=nat, in_=ap.rearrange("(a b) -> a b", b=P))
            ps = pspool.tile([P, DO], f32, tag="lncol_ps")
            nc.tensor.transpose(ps, nat, ident[:DO, :DO])
            t = consts.tile([P, DO], f32, tag=f"lncol_{ap.tensor.name}")
            nc.vector.tensor_copy(out=t, in_=ps)
            return t

        # mask bias column [P, nkb]: 0 where mask==1, -100 where mask==0
        with tc.tile_pool(name="ps_init", bufs=2, space="PSUM") as ps_init:
            mask_nat = small.tile([nkb, P], mybir.dt.int32, tag="mask_nat")
            nc.sync.dma_start(out=mask_nat,
                              in_=Md.rearrange("(a b) -> a b", b=P))
            mask_f = small.tile([nkb, P], f32, tag="mask_f")
            nc.vector.tensor_copy(out=mask_f, in_=mask_nat)  # int -> float cast
            mask_ps = ps_init.tile([P, nkb], f32, tag="mask_ps")
            nc.tensor.transpose(mask_ps, mask_f, ident[:nkb, :nkb])
            mb = consts.tile([P, nkb], f32)
            # (m - 1) * 100 : 1 -> 0, 0 -> -100
            nc.vector.tensor_scalar(out=mb, in0=mask_ps,
                                    scalar1=-1.0, scalar2=100.0,
                                    op0=OP.add, op1=OP.mult)

            g0T = load_colT(g0d, ps_init) if apply_g0b0 else None
            b0T = load_colT(b0d, ps_init) if apply_g0b0 else None
            g1T = load_colT(g1d, ps_init) if apply_g1b1 else None
            b1T = load_colT(b1d, ps_init) if apply_g1b1 else None
            bqT = load_colT(bqd, ps_init)
            bkT = load_colT(bkd, ps_init)
            boT = load_colT(bod, ps_init)


        # ---------- phase B: projections ----------
        qT = bigs.tile([P, DO, NQ], f32, tag="buf_qproj")
        qTb = bigs.tile([P, DO, NQ], BF16, tag="buf_qproj_bf")
        kT = bigs.tile([P, DO, NKC], BF16, tag="buf_kproj")
        vA = bigs.tile([P, nkb, H * (HD + 1)], BF16, tag="buf_v")

        with tc.tile_pool(name="ps_proj", bufs=8, space="PSUM") as ps_proj:
            # qT[dv, nq] = Wq^T @ Q^T + bq x 1 ; same for kT
            qchunks = [(qc * 512, 512) for qc in range(QC)]
            for WT, XT_src, dstT, dstT2, bT, ch in (
                    (Wq, QT, qT, qTb, bqT, qchunks),
                    (Wk, KT, kT, None, bkT, kchunks)):
                for do in range(DO):
                    for off, w in ch:
                        ps = ps_proj.tile([P, 512], f32, tag="projps")
                        for ko in range(DO):
                            nc.tensor.matmul(
                                ps[:, :w],
                                lhsT=_mm(WT[:, ko, do * P:(do + 1) * P]),
                                rhs=_mm(XT_src[:, ko, off:off + w]),
                                start=(ko == 0), stop=(ko == DO - 1))
                        # bias folded into the psum->sbuf copy (per-partition).
                        # kT is stored bf16 (scores-matmul operand); qT keeps
                        # an fp32 copy for the residual plus a bf16 copy.
                        dsl = dstT[:, do, off:off + w]
                        nc.vector.tensor_scalar_add(
                            out=dsl if dstT.dtype == BF16 else _mo(dsl),
                            in0=ps[:, :w], scalar1=bT[:, do:do + 1])
                        if dstT2 is not None:
                            nc.vector.tensor_scalar_add(
                                out=dstT2[:, do, off:off + w],
                                in0=ps[:, :w], scalar1=bT[:, do:do + 1])
            # v[nk, dv] = K @ Wv + 1 x bv, stored augmented per head:
            # vA[:, no, h*65 : h*65+64] = v columns of head h, vA[.., h*65+64] = 1.
            # The ones column makes the U matmul also produce the softmax
            # denominator in psum row 64 (fp32r matmuls must write base
            # partition 0, so the denominator must ride along, not col-pack).
            for no in range(nkb):
                ps = ps_proj.tile([P, 512], f32, tag="projps")
                for ko in range(DO):
                    nc.tensor.matmul(
                        ps,
                        lhsT=_mm(KT[:, ko, no * P:(no + 1) * P]),
                        rhs=_mm(Wv[:, ko, :]),
                        start=(ko == 0), stop=False)
                nc.tensor.matmul(
                    ps, lhsT=_mm(ones_row[:, :P]), rhs=_mm(bv_r),
                    start=False, stop=True)
                va = vA[:, no, :].rearrange("p (h e) -> p h e", e=HD + 1)
                nc.scalar.copy(out=va[:, :, :HD],
                               in_=ps.rearrange("p (h e) -> p h e", e=HD))
                nc.vector.tensor_copy(
                    out=va[:, :, HD:HD + 1],
                    in_=ones_stage[:, :H].rearrange("p (a b) -> p a b", b=1))

        # ---------- phase C: attention (head pairs) ----------
        # ZT = qT + attn @ v   (unnormalized accumulate, then divide by rowsum)
        ZT = bigs.tile([P, DO, NQ], f32, tag="buf_zt")
        SCALE = 1.0 / np.sqrt(np.float32(D))

        # fp32r matmuls may only write PSUM at base partition 0, so each
        # head accumulates U (attn@v) in rows 0:64 of its own psum tile; the
        # augmented ones-column of vA makes row 64 the softmax denominator.
        # 1/den is broadcast back over 64 partitions by a small SBUF->SBUF
        # DMA (the only cross-partition mover), and odd heads' results are
        # DMA-shifted into partitions 64:128 of ZT.
        with tc.tile_pool(name="ps_att", bufs=2, space="PSUM") as ps_att, \
             tc.tile_pool(name="ps_sc", bufs=2, space="PSUM") as ps_sc, \
             tc.tile_pool(name="pt_pool", bufs=3) as pt_pool, \
             tc.tile_pool(name="den_pool", bufs=2) as den_pool, \
             tc.tile_pool(name="den_dram", bufs=2, space="DRAM") as den_dram:
            for h in range(H):
                hp, hh = divmod(h, 2)
                r0 = HD * hh
                Ups = ps_att.tile([HD + 1, NQ], f32, tag="u", name=f"U{h}")
                for kb in range(nkb):
                    sc = ps_sc.tile([P, NQ], f32, tag="scores",
                                    name=f"S{h}_{kb}")
                    for qc in range(QC):
                        nc.tensor.matmul(
                            sc[:, qc * 512:(qc + 1) * 512],
                            lhsT=kT[r0:r0 + HD, hp, kb * P:(kb + 1) * P],
                            rhs=qTb[r0:r0 + HD, hp, qc * 512:(qc + 1) * 512],
                            start=True, stop=True)
                    # exp((s * scale) + maskbias) ; PSUM -> SBUF (bf16)
                    pt = pt_pool.tile([P, NQ], BF16, tag="pt")
                    nc.scalar.activation(pt, sc, AF.Exp,
                                         bias=mb[:, kb:kb + 1], scale=SCALE)
                    # [U ; den] += [v_h | 1]^T @ p
                    for qc in range(QC):
                        nc.tensor.matmul(
                            Ups[:, qc * 512:(qc + 1) * 512],
                            lhsT=vA[:, kb, h * (HD + 1):(h + 1) * (HD + 1)],
                            rhs=pt[:, qc * 512:(qc + 1) * 512],
                            start=(kb == 0), stop=(kb == nkb - 1))
                # head output: U / den (+ qT residual) into ZT rows r0:r0+64
                rrow = den_pool.tile([HD + 1, NQ], f32, tag="rrow")
                nc.vector.reciprocal(out=rrow[HD:HD + 1, :],
                                     in_=Ups[HD:HD + 1, :])
                # cross-partition broadcast of 1/den: SBUF row -> DRAM -> all
                # 64 partitions (DRAM APs may have stride-0 partition dims)
                dscratch = den_dram.tile([1, NQ], f32, tag="dd")
                nc.sync.dma_start(out=dscratch, in_=rrow[HD:HD + 1, :])
                rec = den_pool.tile([HD, NQ], f32, tag="rec")
                nc.sync.dma_start(out=rec, in_=dscratch.to_broadcast((HD, NQ)))
                if hh == 0:
                    nc.vector.tensor_mul(out=_mo(ZT[:HD, hp, :]),
                                         in0=Ups[:HD, :], in1=rec)
                    nc.gpsimd.tensor_add(out=_mo(ZT[:HD, hp, :]),
                                         in0=ZT[:HD, hp, :],
                                         in1=qT[:HD, hp, :])
                else:
                    tmp = den_pool.tile([HD, NQ], f32, tag="tmp")
                    nc.vector.tensor_mul(out=_mo(tmp), in0=Ups[:HD, :], in1=rec)
                    nc.sync.dma_start(out=_mo(ZT[HD:P, hp, :]), in_=_mo(tmp))
                    nc.gpsimd.tensor_add(out=_mo(ZT[HD:P, hp, :]),
                                         in0=ZT[HD:P, hp, :],
                                         in1=qT[HD:P, hp, :])

        # ---------- layernorm helper (d-major) ----------
        def layernorm(srcT, dstT, gT, bT, ps_pool, sq_pool, st_pool,
                      round_out=False, out_dma=None):
            """dstT = LN(srcT) over the d axis (partitions+groups)."""
            for qc in range(QC):
                s1 = ps_pool.tile([P, 512], f32, tag="s1")
                s2 = ps_pool.tile([P, 512], f32, tag="s2")
                for do in range(DO):
                    nc.tensor.matmul(
                        s1, lhsT=_mm(ones_pp), rhs=_mm(srcT[:, do, qc * 512:(qc + 1) * 512]),
                        start=(do == 0), stop=(do == DO - 1))
                for do in range(DO):
                    sq = sq_pool.tile([P, 512], f32, tag="sq")
                    nc.vector.tensor_mul(out=_mo(sq),
                                         in0=srcT[:, do, qc * 512:(qc + 1) * 512],
                                         in1=srcT[:, do, qc * 512:(qc + 1) * 512])
                    nc.tensor.matmul(s2, lhsT=_mm(ones_pp), rhs=_mm(sq),
                                     start=(do == 0), stop=(do == DO - 1))
                mu = st_pool.tile([P, 512], f32, tag="mu")
                nc.vector.tensor_scalar_mul(out=mu, in0=s1, scalar1=1.0 / D)
                ex2 = st_pool.tile([P, 512], f32, tag="ex2")
                nc.vector.tensor_scalar_mul(out=ex2, in0=s2, scalar1=1.0 / D)
                musq = st_pool.tile([P, 512], f32, tag="musq")
                nc.scalar.square(out=musq, in_=mu)
                var = st_pool.tile([P, 512], f32, tag="var")
                nc.vector.tensor_sub(out=var, in0=ex2, in1=musq)
                sd = st_pool.tile([P, 512], f32, tag="sd")
                nc.scalar.activation(sd, var, AF.Sqrt, bias=epsT)
                rstd = st_pool.tile([P, 512], f32, tag="rstd")
                nc.vector.reciprocal(out=rstd, in_=sd)
                for do in range(DO):
                    dslice = dstT[:, do, qc * 512:(qc + 1) * 512]
                    sslice = srcT[:, do, qc * 512:(qc + 1) * 512]
                    ro = _mo if round_out else (lambda x: x)
                    nc.gpsimd.tensor_sub(out=ro(dslice), in0=sslice, in1=mu)
                    nc.vector.tensor_mul(out=ro(dslice), in0=dslice, in1=rstd)
                    if gT is not None:
                        nc.vector.tensor_scalar(
                            out=ro(dslice), in0=dslice,
                            scalar1=gT[:, do:do + 1], scalar2=bT[:, do:do + 1],
                            op0=OP.mult, op1=OP.add)
                    if out_dma is not None:
                        nc.sync.dma_start(out=out_dma[:, do, qc * 512:(qc + 1) * 512],
                                          in_=dslice)

        # ---------- phase D: LN0 ----------
        XT = bigs.tile([P, DO, NQ], f32, tag="buf_kt")  # reuse KT slot
        with tc.tile_pool(name="ps_ln0", bufs=2, space="PSUM") as ps_ln0, \
             tc.tile_pool(name="sq0", bufs=3) as sq0, \
             tc.tile_pool(name="st0", bufs=2) as st0:
            layernorm(ZT, XT, g0T, b0T, ps_ln0, sq0, st0, round_out=True)

        # ---------- phase E: FC + relu + residual ----------
        Z2 = bigs.tile([P, DO, NQ], f32, tag="buf_qt")  # reuse QT slot
        with tc.tile_pool(name="ps_fc", bufs=8, space="PSUM") as ps_fc, \
             tc.tile_pool(name="fc_sb", bufs=3) as fc_sb:
            for do in range(DO):
                for qc in range(QC):
                    ps = ps_fc.tile([P, 512], f32, tag="fcps")
                    for ko in range(DO):
                        nc.tensor.matmul(
                            ps,
                            lhsT=_mm(Wo[:, ko, do * P:(do + 1) * P]),
                            rhs=_mm(XT[:, ko, qc * 512:(qc + 1) * 512]),
                            start=(ko == 0), stop=(ko == DO - 1))
                    rel = fc_sb.tile([P, 512], f32, tag="rel")
                    nc.scalar.activation(rel, ps, AF.Relu,
                                         bias=boT[:, do:do + 1])
                    nc.vector.tensor_add(
                        out=_mo(Z2[:, do, qc * 512:(qc + 1) * 512]),
                        in0=rel,
                        in1=XT[:, do, qc * 512:(qc + 1) * 512])

        # ---------- phase F: LN1 -> output ----------
        OT = bigs.tile([P, DO, NQ], f32, tag="buf_zt")  # reuse ZT slot
        with tc.tile_pool(name="ps_ln1", bufs=2, space="PSUM") as ps_ln1, \
             tc.tile_pool(name="sq1", bufs=3) as sq1, \
             tc.tile_pool(name="st1", bufs=2) as st1:
            layernorm(Z2, OT, g1T, b1T, ps_ln1, sq1, st1,
                      out_dma=OTd.rearrange("(o p) q -> p o q", p=P))


# ------------------------------------------------------------------
# host-side entry point
# ------------------------------------------------------------------
_CACHE = {}


def _get_program(repeat, apply_g0b0, apply_g1b1, nkb=NO):
    key = (repeat, apply_g0b0, apply_g1b1, nkb)
    if key not in _CACHE:
        _CACHE[key] = build_program(repeat, apply_g0b0, apply_g1b1, nkb)
    return _CACHE[key]


def compact_keys(K_b, mask_b, nkb):
    """Move unmasked keys to the front (order-preserving) and truncate to
    nkb*128 rows.  Softmax over keys is permutation-invariant and fully
    masked keys contribute exactly zero, so this is output-preserving as
    long as all unmasked keys survive the truncation."""
    nkc = nkb * P
    order = np.argsort(mask_b == 0, kind="stable")[:nkc]
    return (np.ascontiguousarray(K_b[order]),
            np.ascontiguousarray(mask_b[order]))


def pick_nkb(attention_mask):
    counts = (np.asarray(attention_mask) != 0).sum(axis=-1)
    return max(1, min(NO, int(-(-int(counts.max()) // P))))


def make_in_maps(inputs, nkb):
    shared = {k: np.asarray(inputs[k], np.float32)
              for k in ("Wq", "Wk", "Wv", "Wo", "bq", "bk", "bv", "bo",
                        "g0", "b0", "g1", "b1")}
    Q = np.asarray(inputs["Q"], np.float32)
    K = np.asarray(inputs["K"], np.float32)
    mask = np.asarray(inputs["attention_mask"], np.int32)
    in_maps = []
    for b in range(B):
        m = dict(shared)
        m["Q"] = np.ascontiguousarray(Q[b])
        Kc, mc = compact_keys(K[b], mask[b], nkb)
        m["K"] = Kc
        m["attention_mask"] = mc
        in_maps.append(m)
    return in_maps


def kernel(Q, K, attention_mask, Wq, bq, Wk, bk, Wv, bv, Wo, bo,
           g0, b0, g1, b1, _repeat=1):
    inputs = {
        "Q": Q, "K": K, "attention_mask": attention_mask,
        "Wq": Wq, "bq": bq, "Wk": Wk, "bk": bk, "Wv": Wv, "bv": bv,
        "Wo": Wo, "bo": bo, "g0": g0, "b0": b0, "g1": g1, "b1": b1,
    }
    apply_g0b0 = not (np.all(np.asarray(g0) == 1.0)
                      and np.all(np.asarray(b0) == 0.0))
    apply_g1b1 = not (np.all(np.asarray(g1) == 1.0)
                      and np.all(np.asarray(b1) == 0.0))
    nkb = pick_nkb(attention_mask)
    nc = _get_program(_repeat, apply_g0b0, apply_g1b1, nkb)
    in_maps = make_in_maps(inputs, nkb)

    res = bass_utils.run_bass_kernel_spmd(
        nc, in_maps, core_ids=list(range(N_CORES)), trace=False)

    out = np.empty((B, NQ, D), np.float32)
    for b in range(B):
        out[b] = res.results[b]["OT"].T
    return out


# revision 32
# speedup vs baseline: 1.0300x; 1.0300x over previous
"""Trainium2 Bass kernel for nn_MAB_72911364817388 (dense transformer block).

Reference computation (per batch element b):
    q = Q @ Wq + bq ; k = K @ Wk + bk ; v = K @ Wv + bv        (1024x512 @ 512x512)
    scores = einsum("qhd,khd->hqk", qh, kh) / sqrt(512)
    scores = where(mask==0, -1e4, scores); attn = softmax(scores, axis=k)
    oh = qh + attn @ vh ; O = LN0(oh) ; O = O + relu(O @ Wo + bo) ; O = LN1(O)

Strategy: pure data-parallel over batch B=8 -> one batch element per core.
All on-chip activations are kept "d-major" (feature dim on partitions),
which makes attention, the FC layer and per-feature bias/scale natural.
LayerNorm reductions over d (the partition axis) are done with ones-vector
matmuls on the TensorEngine, which also replicates the stats across all
128 partitions for free.

Softmax is computed unnormalized: p = exp(s/sqrt(512) + maskbias), where
maskbias is -100 for masked keys (exp underflows to ~3e-44, matching the
reference's -1e4 masking after normalization to < 1e-40 relative).  The
denominator comes from an extra ones-column matmul and is divided out
after attn @ v.  Scores never exceed ~±6 so no max-subtraction is needed.

Matmuls run as float32r (fp32 storage, reduced-precision PE mode, 4x
faster than plain fp32; ~1.6e-4 rel err per matmul measured on HW).  The
attention core (scores and attn@v) uses bf16 operands, which stream at
the same rate but with much cheaper self-loading weights (~237ns vs
~329ns per 512-wide matmul measured).  Keys are host-compacted: unmasked
keys are moved to the front (softmax is permutation-invariant; fully
masked keys contribute exactly zero), so the attention core processes
~640 of 1024 keys for the usual p=0.5 masks.  Measured end-to-end
relative error vs the jax reference: ~6e-4 (scale-relative max).
"""

import numpy as np

import concourse.bass as bass
import concourse.mybir as mybir
import concourse.tile as tile
from concourse import bacc, bass_utils
from concourse.masks import make_identity

# Problem shapes (hardcoded per contract).
B = 8
NQ = NK = 1024
D = 512  # DQ = DK = DV
H = 8
HD = 64
P = 128
EPS = 1e-5
N_CORES = 8

DO = D // P  # 4   d-major partition groups
NO = NQ // P  # 8  q/k-major partition groups
QC = NQ // 512  # 2 free-dim chunks of 512

F32 = mybir.dt.float32
BF16 = mybir.dt.bfloat16
MM_DT = mybir.dt.float32r

AF = mybir.ActivationFunctionType
OP = mybir.AluOpType


def _mm(a):
    """Bitcast an fp32 AP to the matmul dtype (consumer side)."""
    return a.bitcast(MM_DT) if MM_DT != F32 else a


def _mo(a):
    """Bitcast a producer's out AP to the matmul dtype, so the BIR verifier
    sees matmul inputs as produced-rounded fp32r."""
    return a.bitcast(MM_DT) if MM_DT != F32 else a


def build_program(repeat: int = 1, apply_g0b0: bool = True,
                  apply_g1b1: bool = True, nkb: int = NO):
    """Build the per-core Bass program for nkb 128-row key blocks.

    The host compacts unmasked keys to the front (softmax is permutation-
    invariant over keys, and fully-masked keys contribute exactly 0), so
    nkb is usually ceil(max_unmasked/128) ~ 5 instead of 8."""
    nc = bacc.Bacc("TRN2", target_bir_lowering=False, debug=False,
                   num_devices=N_CORES)

    NKC = nkb * P
    Qd = nc.dram_tensor("Q", [NQ, D], F32, kind="ExternalInput").ap()
    Kd = nc.dram_tensor("K", [NKC, D], F32, kind="ExternalInput").ap()
    Md = nc.dram_tensor("attention_mask", [NKC], mybir.dt.int32,
                        kind="ExternalInput").ap()
    Wqd = nc.dram_tensor("Wq", [D, D], F32, kind="ExternalInput").ap()
    Wkd = nc.dram_tensor("Wk", [D, D], F32, kind="ExternalInput").ap()
    Wvd = nc.dram_tensor("Wv", [D, D], F32, kind="ExternalInput").ap()
    Wod = nc.dram_tensor("Wo", [D, D], F32, kind="ExternalInput").ap()
    bqd = nc.dram_tensor("bq", [D], F32, kind="ExternalInput").ap()
    bkd = nc.dram_tensor("bk", [D], F32, kind="ExternalInput").ap()
    bvd = nc.dram_tensor("bv", [D], F32, kind="ExternalInput").ap()
    bod = nc.dram_tensor("bo", [D], F32, kind="ExternalInput").ap()
    g0d = nc.dram_tensor("g0", [D], F32, kind="ExternalInput").ap()
    b0d = nc.dram_tensor("b0", [D], F32, kind="ExternalInput").ap()
    g1d = nc.dram_tensor("g1", [D], F32, kind="ExternalInput").ap()
    b1d = nc.dram_tensor("b1", [D], F32, kind="ExternalInput").ap()
    # Output is O^T (d-major); the host transposes back.
    OTd = nc.dram_tensor("OT", [D, NQ], F32, kind="ExternalOutput").ap()

    with tile.TileContext(nc) as tc:
        def body():
            _build_body(nc, tc,
                        Qd, Kd, Md, Wqd, Wkd, Wvd, Wod,
                        bqd, bkd, bvd, bod, g0d, b0d, g1d, b1d, OTd,
                        apply_g0b0, apply_g1b1, nkb)

        if repeat == 1:
            body()
        else:
            with tc.For_i(0, repeat, 1,
                          hint_engines=(mybir.EngineType.PE,
                                        mybir.EngineType.Activation,
                                        mybir.EngineType.DVE,
                                        mybir.EngineType.SP,
                                        mybir.EngineType.Pool)):
                body()

    nc.compile()
    return nc


def _build_body(nc, tc, Qd, Kd, Md, Wqd, Wkd, Wvd, Wod,
                bqd, bkd, bvd, bod, g0d, b0d, g1d, b1d, OTd,
                apply_g0b0, apply_g1b1, nkb):
    f32 = F32
    NKC = nkb * P
    kchunks = []
    off = 0
    while off < NKC:
        w = min(512, NKC - off)
        kchunks.append((off, w))
        off += w
    import contextlib
    ctx = contextlib.ExitStack()
    with ctx:
        consts = ctx.enter_context(tc.tile_pool(name="consts", bufs=1))
        bigs = ctx.enter_context(tc.tile_pool(name="bigs", bufs=1))
        small = ctx.enter_context(tc.tile_pool(name="small", bufs=3))

        # ---------- constants ----------
        ident = consts.tile([P, P], f32)
        make_identity(nc, ident)
        ones_stage = consts.tile([P, 512], f32)
        nc.vector.memset(ones_stage, 1.0)
        ones_pp = consts.tile([P, P], f32)   # all-ones for LN stat matmuls
        nc.vector.tensor_copy(out=_mo(ones_pp), in_=ones_stage[:, :P])
        ones_row = consts.tile([1, 512], f32)  # ones moving-vector for bias rank-1
        nc.vector.tensor_copy(out=_mo(ones_row), in_=ones_stage[:1, :])
        epsT = consts.tile([P, 1], f32)
        nc.vector.memset(epsT, EPS)

        # ---------- phase A: load + transpose Q, K; weight DMAs interleave
        Wq = consts.tile([P, DO, D], f32)
        Wk = consts.tile([P, DO, D], f32)
        Wv = consts.tile([P, DO, D], f32)
        Wo = consts.tile([P, DO, D], f32)
        QT = bigs.tile([P, DO, NQ], f32, tag="buf_qt")
        KT = bigs.tile([P, DO, NKC], f32, tag="buf_kt")
        with tc.tile_pool(name="ps_tr", bufs=4, space="PSUM") as ps_tr, \
             tc.tile_pool(name="rawp", bufs=3) as rawp:
            # batched raw loads: fewer, bigger DMAs (latency amortized)
            q0 = rawp.tile([P, 4, D], f32, tag="qraw")
            nc.sync.dma_start(out=q0,
                              in_=Qd[:512, :].rearrange("(j p) d -> p j d", p=P))
            nc.sync.dma_start(out=_mo(Wq),
                              in_=_mo(Wqd.rearrange("(o p) n -> p o n", p=P)))
            kr = rawp.tile([P, nkb, D], f32, tag="kraw")
            nc.sync.dma_start(out=kr,
                              in_=Kd.rearrange("(j p) d -> p j d", p=P))
            q1 = rawp.tile([P, 4, D], f32, tag="qraw")
            nc.sync.dma_start(out=q1,
                              in_=Qd[512:, :].rearrange("(j p) d -> p j d", p=P))
            nc.sync.dma_start(out=_mo(Wk),
                              in_=_mo(Wkd.rearrange("(o p) n -> p o n", p=P)))
            nc.sync.dma_start(out=_mo(Wv),
                              in_=_mo(Wvd.rearrange("(o p) n -> p o n", p=P)))
            nc.sync.dma_start(out=_mo(Wo),
                              in_=_mo(Wod.rearrange("(o p) n -> p o n", p=P)))

            worklist = [(j, 0, q0) for j in range(4)]
            worklist += [(j, 1, kr) for j in range(nkb)]
            worklist += [(j + 4, 0, q1) for j in range(4)]
            for no, sd, rawt in worklist:
                dstT, ceng = ((QT, 0), (KT, 1))[sd]
                raw = rawt[:, no % 4 if sd == 0 else no, :]
                ps = ps_tr.tile([P, 512], f32, tag="trps")
                for do in range(DO):
                    nc.tensor.transpose(ps[:, do * P:(do + 1) * P],
                                        raw[:, do * P:(do + 1) * P], ident)
                dst = dstT[:, :, no * P:(no + 1) * P]
                psv = ps.rearrange("p (a b) -> p a b", b=P)
                if (no + ceng) % 2 == 0:
                    nc.vector.tensor_copy(out=_mo(dst), in_=psv)
                else:
                    nc.scalar.copy(out=_mo(dst), in_=psv)

        # bias rows [1, 512]
        def load_row(ap):
            t = consts.tile([1, 512], f32)
            nc.sync.dma_start(out=_mo(t), in_=_mo(ap[None, :]))
            return t
        bv_r = load_row(bvd)

        # LN scale/shift per-partition columns [P, DO] (only if non-identity)
        def load_colT(ap, pspool):
            # [512] -> sbuf [4,128] -> PE transpose -> [128,4]
            nat = small.tile([DO, P], f32, tag="lncol_nat")
            nc.sync.dma_start(out=nat, in_=ap.rearrange("(a b) -> a b", b=P))
            ps = pspool.tile([P, DO], f32, tag="lncol_ps")
            nc.tensor.transpose(ps, nat, ident[:DO, :DO])
            t = consts.tile([P, DO], f32, tag=f"lncol_{ap.tensor.name}")
            nc.vector.tensor_copy(out=t, in_=ps)
            return t

        # mask bias column [P, nkb]: 0 where mask==1, -100 where mask==0
        with tc.tile_pool(name="ps_init", bufs=2, space="PSUM") as ps_init:
            mask_nat = small.tile([nkb, P], mybir.dt.int32, tag="mask_nat")
            nc.sync.dma_start(out=mask_nat,
                              in_=Md.rearrange("(a b) -> a b", b=P))
            mask_f = small.tile([nkb, P], f32, tag="mask_f")
            nc.vector.tensor_copy(out=mask_f, in_=mask_nat)  # int -> float cast
            mask_ps = ps_init.tile([P, nkb], f32, tag="mask_ps")
            nc.tensor.transpose(mask_ps, mask_f, ident[:nkb, :nkb])
            mb = consts.tile([P, nkb], f32)
            # (m - 1) * 100 : 1 -> 0, 0 -> -100
            nc.vector.tensor_scalar(out=mb, in0=mask_ps,
                                    scalar1=-1.0, scalar2=100.0,
                                    op0=OP.add, op1=OP.mult)

            g0T = load_colT(g0d, ps_init) if apply_g0b0 else None
            b0T = load_colT(b0d, ps_init) if apply_g0b0 else None
            g1T = load_colT(g1d, ps_init) if apply_g1b1 else None
            b1T = load_colT(b1d, ps_init) if apply_g1b1 else None
            bqT = load_colT(bqd, ps_init)
            bkT = load_colT(bkd, ps_init)
            boT = load_colT(bod, ps_init)


        # ---------- phase B: projections ----------
        qT = bigs.tile([P, DO, NQ], f32, tag="buf_qproj")
        qTb = bigs.tile([P, DO, NQ], BF16, tag="buf_qproj_bf")
        kT = bigs.tile([P, DO, NKC], BF16, tag="buf_kproj")
        vA = bigs.tile([P, nkb, H * (HD + 1)], BF16, tag="buf_v")

        with tc.tile_pool(name="ps_proj", bufs=8, space="PSUM") as ps_proj:
            # qT[dv, nq] = Wq^T @ Q^T + bq x 1 ; same for kT
            qchunks = [(qc * 512, 512) for qc in range(QC)]
            for WT, XT_src, dstT, dstT2, bT, ch in (
                    (Wq, QT, qT, qTb, bqT, qchunks),
                    (Wk, KT, kT, None, bkT, kchunks)):
                for do in range(DO):
                    for off, w in ch:
                        ps = ps_proj.tile([P, 512], f32, tag="projps")
                        for ko in range(DO):
                            nc.tensor.matmul(
                                ps[:, :w],
                                lhsT=_mm(WT[:, ko, do * P:(do + 1) * P]),
                                rhs=_mm(XT_src[:, ko, off:off + w]),
                                start=(ko == 0), stop=(ko == DO - 1))
                        # bias folded into the psum->sbuf copy (per-partition).
                        # kT is stored bf16 (scores-matmul operand); qT keeps
                        # an fp32 copy for the residual plus a bf16 copy.
                        dsl = dstT[:, do, off:off + w]
                        nc.vector.tensor_scalar_add(
                            out=dsl if dstT.dtype == BF16 else _mo(dsl),
                            in0=ps[:, :w], scalar1=bT[:, do:do + 1])
                        if dstT2 is not None:
                            nc.vector.tensor_scalar_add(
                                out=dstT2[:, do, off:off + w],
                                in0=ps[:, :w], scalar1=bT[:, do:do + 1])
            # v[nk, dv] = K @ Wv + 1 x bv, stored augmented per head:
            # vA[:, no, h*65 : h*65+64] = v columns of head h, vA[.., h*65+64] = 1.
            # The ones column makes the U matmul also produce the softmax
            # denominator in psum row 64 (fp32r matmuls must write base
            # partition 0, so the denominator must ride along, not col-pack).
            for no in range(nkb):
                ps = ps_proj.tile([P, 512], f32, tag="projps")
                for ko in range(DO):
                    nc.tensor.matmul(
                        ps,
                        lhsT=_mm(KT[:, ko, no * P:(no + 1) * P]),
                        rhs=_mm(Wv[:, ko, :]),
                        start=(ko == 0), stop=False)
                nc.tensor.matmul(
                    ps, lhsT=_mm(ones_row[:, :P]), rhs=_mm(bv_r),
                    start=False, stop=True)
                va = vA[:, no, :].rearrange("p (h e) -> p h e", e=HD + 1)
                nc.scalar.copy(out=va[:, :, :HD],
                               in_=ps.rearrange("p (h e) -> p h e", e=HD))
                nc.vector.tensor_copy(
                    out=va[:, :, HD:HD + 1],
                    in_=ones_stage[:, :H].rearrange("p (a b) -> p a b", b=1))

        # ---------- phase C: attention (head pairs) ----------
        # ZT = qT + attn @ v   (unnormalized accumulate, then divide by rowsum)
        ZT = bigs.tile([P, DO, NQ], f32, tag="buf_zt")
        SCALE = 1.0 / np.sqrt(np.float32(D))

        # fp32r matmuls may only write PSUM at base partition 0, so each
        # head accumulates U (attn@v) in rows 0:64 of its own psum tile; the
        # augmented ones-column of vA makes row 64 the softmax denominator.
        # 1/den is broadcast back over 64 partitions by a small SBUF->SBUF
        # DMA (the only cross-partition mover), and odd heads' results are
        # DMA-shifted into partitions 64:128 of ZT.
        with tc.tile_pool(name="ps_att", bufs=2, space="PSUM") as ps_att, \
             tc.tile_pool(name="ps_sc", bufs=2, space="PSUM") as ps_sc, \
             tc.tile_pool(name="pt_pool", bufs=4) as pt_pool, \
             tc.tile_pool(name="den_pool", bufs=3) as den_pool, \
             tc.tile_pool(name="den_dram", bufs=3, space="DRAM") as den_dram:
            for h in range(H):
                hp, hh = divmod(h, 2)
                r0 = HD * hh
                Ups = ps_att.tile([HD + 1, NQ], f32, tag="u", name=f"U{h}")
                for kb in range(nkb):
                    sc = ps_sc.tile([P, NQ], f32, tag="scores",
                                    name=f"S{h}_{kb}")
                    for qc in range(QC):
                        nc.tensor.matmul(
                            sc[:, qc * 512:(qc + 1) * 512],
                            lhsT=kT[r0:r0 + HD, hp, kb * P:(kb + 1) * P],
                            rhs=qTb[r0:r0 + HD, hp, qc * 512:(qc + 1) * 512],
                            start=True, stop=True)
                    # exp((s * scale) + maskbias) ; PSUM -> SBUF (bf16)
                    pt = pt_pool.tile([P, NQ], BF16, tag="pt")
                    nc.scalar.activation(pt, sc, AF.Exp,
                                         bias=mb[:, kb:kb + 1], scale=SCALE)
                    # [U ; den] += [v_h | 1]^T @ p
                    for qc in range(QC):
                        nc.tensor.matmul(
                            Ups[:, qc * 512:(qc + 1) * 512],
                            lhsT=vA[:, kb, h * (HD + 1):(h + 1) * (HD + 1)],
                            rhs=pt[:, qc * 512:(qc + 1) * 512],
                            start=(kb == 0), stop=(kb == nkb - 1))
                # head output: U / den (+ qT residual) into ZT rows r0:r0+64
                rrow = den_pool.tile([HD + 1, NQ], f32, tag="rrow")
                nc.vector.reciprocal(out=rrow[HD:HD + 1, :],
                                     in_=Ups[HD:HD + 1, :])
                # cross-partition broadcast of 1/den: SBUF row -> DRAM -> all
                # 64 partitions (DRAM APs may have stride-0 partition dims)
                dscratch = den_dram.tile([1, NQ], f32, tag="dd")
                nc.sync.dma_start(out=dscratch, in_=rrow[HD:HD + 1, :])
                rec = den_pool.tile([HD, NQ], f32, tag="rec")
                nc.sync.dma_start(out=rec, in_=dscratch.to_broadcast((HD, NQ)))
                if hh == 0:
                    nc.vector.tensor_mul(out=_mo(ZT[:HD, hp, :]),
                                         in0=Ups[:HD, :], in1=rec)
                    nc.gpsimd.tensor_add(out=_mo(ZT[:HD, hp, :]),
                                         in0=ZT[:HD, hp, :],
                                         in1=qT[:HD, hp, :])
                else:
                    tmp = den_pool.tile([HD, NQ], f32, tag="tmp")
                    nc.vector.tensor_mul(out=_mo(tmp), in0=Ups[:HD, :], in1=rec)
                    nc.sync.dma_start(out=_mo(ZT[HD:P, hp, :]), in_=_mo(tmp))
                    nc.gpsimd.tensor_add(out=_mo(ZT[HD:P, hp, :]),
                                         in0=ZT[HD:P, hp, :],
                                         in1=qT[HD:P, hp, :])

        # ---------- layernorm helper (d-major) ----------
        def layernorm(srcT, dstT, gT, bT, ps_pool, sq_pool, st_pool,
                      round_out=False, out_dma=None):
            """dstT = LN(srcT) over the d axis (partitions+groups)."""
            for qc in range(QC):
                s1 = ps_pool.tile([P, 512], f32, tag="s1")
                s2 = ps_pool.tile([P, 512], f32, tag="s2")
                for do in range(DO):
                    nc.tensor.matmul(
                        s1, lhsT=_mm(ones_pp), rhs=_mm(srcT[:, do, qc * 512:(qc + 1) * 512]),
                        start=(do == 0), stop=(do == DO - 1))
                for do in range(DO):
                    sq = sq_pool.tile([P, 512], f32, tag="sq")
                    nc.vector.tensor_mul(out=_mo(sq),
                                         in0=srcT[:, do, qc * 512:(qc + 1) * 512],
                                         in1=srcT[:, do, qc * 512:(qc + 1) * 512])
                    nc.tensor.matmul(s2, lhsT=_mm(ones_pp), rhs=_mm(sq),
                                     start=(do == 0), stop=(do == DO - 1))
                mu = st_pool.tile([P, 512], f32, tag="mu")
                nc.vector.tensor_scalar_mul(out=mu, in0=s1, scalar1=1.0 / D)
                ex2 = st_pool.tile([P, 512], f32, tag="ex2")
                nc.vector.tensor_scalar_mul(out=ex2, in0=s2, scalar1=1.0 / D)
                musq = st_pool.tile([P, 512], f32, tag="musq")
                nc.scalar.square(out=musq, in_=mu)
                var = st_pool.tile([P, 512], f32, tag="var")
                nc.vector.tensor_sub(out=var, in0=ex2, in1=musq)
                sd = st_pool.tile([P, 512], f32, tag="sd")
                nc.scalar.activation(sd, var, AF.Sqrt, bias=epsT)
                rstd = st_pool.tile([P, 512], f32, tag="rstd")
                nc.vector.reciprocal(out=rstd, in_=sd)
                for do in range(DO):
                    dslice = dstT[:, do, qc * 512:(qc + 1) * 512]
                    sslice = srcT[:, do, qc * 512:(qc + 1) * 512]
                    ro = _mo if round_out else (lambda x: x)
                    nc.gpsimd.tensor_sub(out=ro(dslice), in0=sslice, in1=mu)
                    nc.vector.tensor_mul(out=ro(dslice), in0=dslice, in1=rstd)
                    if gT is not None:
                        nc.vector.tensor_scalar(
                            out=ro(dslice), in0=dslice,
                            scalar1=gT[:, do:do + 1], scalar2=bT[:, do:do + 1],
                            op0=OP.mult, op1=OP.add)
                    if out_dma is not None:
                        nc.sync.dma_start(out=out_dma[:, do, qc * 512:(qc + 1) * 512],
                                          in_=dslice)

        # ---------- phase D: LN0 ----------
        XT = bigs.tile([P, DO, NQ], f32, tag="buf_kt")  # reuse KT slot
        with tc.tile_pool(name="ps_ln0", bufs=2, space="PSUM") as ps_ln0, \
             tc.tile_pool(name="sq0", bufs=4) as sq0, \
             tc.tile_pool(name="st0", bufs=2) as st0:
            layernorm(ZT, XT, g0T, b0T, ps_ln0, sq0, st0, round_out=True)

        # ---------- phase E: FC + relu + residual ----------
        Z2 = bigs.tile([P, DO, NQ], f32, tag="buf_qt")  # reuse QT slot
        with tc.tile_pool(name="ps_fc", bufs=8, space="PSUM") as ps_fc, \
             tc.tile_pool(name="fc_sb", bufs=3) as fc_sb:
            for do in range(DO):
                for qc in range(QC):
                    ps = ps_fc.tile([P, 512], f32, tag="fcps")
                    for ko in range(DO):
                        nc.tensor.matmul(
                            ps,
                            lhsT=_mm(Wo[:, ko, do * P:(do + 1) * P]),
                            rhs=_mm(XT[:, ko, qc * 512:(qc + 1) * 512]),
                            start=(ko == 0), stop=(ko == DO - 1))
                    rel = fc_sb.tile([P, 512], f32, tag="rel")
                    nc.scalar.activation(rel, ps, AF.Relu,
                                         bias=boT[:, do:do + 1])
                    nc.vector.tensor_add(
                        out=_mo(Z2[:, do, qc * 512:(qc + 1) * 512]),
                        in0=rel,
                        in1=XT[:, do, qc * 512:(qc + 1) * 512])

        # ---------- phase F: LN1 -> output ----------
        OT = bigs.tile([P, DO, NQ], f32, tag="buf_zt")  # reuse ZT slot
        with tc.tile_pool(name="ps_ln1", bufs=2, space="PSUM") as ps_ln1, \
             tc.tile_pool(name="sq1", bufs=4) as sq1, \
             tc.tile_pool(name="st1", bufs=2) as st1:
            layernorm(Z2, OT, g1T, b1T, ps_ln1, sq1, st1,
                      out_dma=OTd.rearrange("(o p) q -> p o q", p=P))


# ------------------------------------------------------------------
# host-side entry point
# ------------------------------------------------------------------
_CACHE = {}


def _get_program(repeat, apply_g0b0, apply_g1b1, nkb=NO):
    key = (repeat, apply_g0b0, apply_g1b1, nkb)
    if key not in _CACHE:
        _CACHE[key] = build_program(repeat, apply_g0b0, apply_g1b1, nkb)
    return _CACHE[key]


def compact_keys(K_b, mask_b, nkb):
    """Move unmasked keys to the front (order-preserving) and truncate to
    nkb*128 rows.  Softmax over keys is permutation-invariant and fully
    masked keys contribute exactly zero, so this is output-preserving as
    long as all unmasked keys survive the truncation."""
    nkc = nkb * P
    order = np.argsort(mask_b == 0, kind="stable")[:nkc]
    return (np.ascontiguousarray(K_b[order]),
            np.ascontiguousarray(mask_b[order]))


def pick_nkb(attention_mask):
    counts = (np.asarray(attention_mask) != 0).sum(axis=-1)
    return max(1, min(NO, int(-(-int(counts.max()) // P))))


def make_in_maps(inputs, nkb):
    shared = {k: np.asarray(inputs[k], np.float32)
              for k in ("Wq", "Wk", "Wv", "Wo", "bq", "bk", "bv", "bo",
                        "g0", "b0", "g1", "b1")}
    Q = np.asarray(inputs["Q"], np.float32)
    K = np.asarray(inputs["K"], np.float32)
    mask = np.asarray(inputs["attention_mask"], np.int32)
    in_maps = []
    for b in range(B):
        m = dict(shared)
        m["Q"] = np.ascontiguousarray(Q[b])
        Kc, mc = compact_keys(K[b], mask[b], nkb)
        m["K"] = Kc
        m["attention_mask"] = mc
        in_maps.append(m)
    return in_maps


def kernel(Q, K, attention_mask, Wq, bq, Wk, bk, Wv, bv, Wo, bo,
           g0, b0, g1, b1, _repeat=1):
    inputs = {
        "Q": Q, "K": K, "attention_mask": attention_mask,
        "Wq": Wq, "bq": bq, "Wk": Wk, "bk": bk, "Wv": Wv, "bv": bv,
        "Wo": Wo, "bo": bo, "g0": g0, "b0": b0, "g1": g1, "b1": b1,
    }
    apply_g0b0 = not (np.all(np.asarray(g0) == 1.0)
                      and np.all(np.asarray(b0) == 0.0))
    apply_g1b1 = not (np.all(np.asarray(g1) == 1.0)
                      and np.all(np.asarray(b1) == 0.0))
    nkb = pick_nkb(attention_mask)
    nc = _get_program(_repeat, apply_g0b0, apply_g1b1, nkb)
    in_maps = make_in_maps(inputs, nkb)

    res = bass_utils.run_bass_kernel_spmd(
        nc, in_maps, core_ids=list(range(N_CORES)), trace=False)

    out = np.empty((B, NQ, D), np.float32)
    for b in range(B):
        out[b] = res.results[b]["OT"].T
    return out
